# revision 3
# baseline (speedup 1.0000x reference)
"""Low-rank linear: out = x @ (U @ V)^T = (x @ V^T) @ U^T on 8 TRN2 cores.

Shapes (hardcoded per problem spec):
  x [4, 2048, 4096] f32 -> flat [8192, 4096], row-sharded 1024 rows/core
  U [4096, 64] f32 (replicated), V [64, 4096] f32 (replicated)
  out [4, 2048, 4096] f32

DMA-bound design: bf16 I/O (half the HBM bytes of f32) and no on-device
transposes -- the host packs x^T into the exact SBUF layout GEMM1 needs:
  XT[h, G, p, j, r] = x_core[h*512 + r, (G*16 + j)*128 + p]   (bf16)
Per core ~16.3 MB of DMA (~46 us at HBM rate) vs ~20 us of PE work.

PE structure (keeps PE well under the DMA roofline even at the cold
1.2 GHz HAM clock):
  GEMM1: col-tiled pairs -- two concurrent matmuls (tile_position (0,0)
    and (0,64)) accumulate partial sums hA = sum over even k-chunks into
    PSUM partitions 0..63 and hB = odd k-chunks into 64..127.
  GEMM2: contracts K=128 over the stacked [hA; hB] against [U^T; U^T]
    (U^T duplicated on both partition halves), so the hA+hB reduction
    happens inside the matmul -- full 128-row array utilization.
Out-DMAs + factor loads ride the scalar HWDGE ring, x loads the sync
ring, so the two rings' transfers overlap.
"""

import sys

for p in ("/opt/trn_rl_repo",):
    if p not in sys.path:
        sys.path.insert(0, p)

import numpy as np
import ml_dtypes

import concourse.bass as bass
import concourse.bacc as bacc_mod
import concourse.mybir as mybir
import concourse.tile as tile
from concourse.bass_utils import run_bass_kernel_spmd

N_CORES = 8
BATCH, SEQ, IN_F = 4, 2048, 4096
ROWS = BATCH * SEQ           # 8192
ROWS_PC = ROWS // N_CORES    # 1024 rows per core
RANK = 64
OUT_F = 4096

P = 128                      # partition dim / k-chunk
N_KC = IN_F // P             # 32 k-chunks
NH = 2                       # row passes per core
RH = ROWS_PC // NH           # 512 rows per pass
NG = 2                       # input DMA chunk-groups per pass (2 MB each)
KCG = N_KC // NG             # 16 k-chunks per chunk-group
N_RB = RH // P               # 4 row-blocks of 128 per pass
NB = 512                     # out-feature block (one PSUM bank of fp32)
PO_W = 2 * NB                # po psum tile spans 2 banks -> 1 copy per 1024
N_PO = OUT_F // PO_W         # 4 po tiles per row-block

F32 = mybir.dt.float32
BF16 = mybir.dt.bfloat16
BF = ml_dtypes.bfloat16


def build_bass():
    nc = bacc_mod.Bacc("TRN2")
    x_d = nc.declare_dram_parameter("XT", [NH, NG, P, KCG * RH], BF16, isOutput=False)
    vt_d = nc.declare_dram_parameter("VT", [P, N_KC * RANK], BF16, isOutput=False)
    u2_d = nc.declare_dram_parameter("U2", [P, OUT_F], BF16, isOutput=False)
    o_d = nc.declare_dram_parameter("out", [NH, 2, P, 2, OUT_F], BF16, isOutput=True)

    with tile.TileContext(nc) as tc:
        with (
            tc.tile_pool(name="const", bufs=1) as const,
            tc.tile_pool(name="xt", bufs=4) as xt_p,
            tc.tile_pool(name="ht", bufs=2) as ht_p,
            tc.tile_pool(name="obuf", bufs=3) as obuf_p,
            tc.tile_pool(name="ph", bufs=2, space="PSUM") as ph_p,
            tc.tile_pool(name="po", bufs=3, space="PSUM") as po_p,
        ):
            vt = const.tile([P, N_KC * RANK], BF16, tag="vt")
            u2 = const.tile([P, OUT_F], BF16, tag="u2")

            xt = {}   # (h, G) -> input tile [P, KCG*RH]
            ph = {}   # h -> GEMM1 psum [P, RH]: rows 0..63 hA, 64..127 hB
            ht = {}   # h -> [hA; hB] in SBUF bf16 [P, RH]
            ob = {}   # (h, q) -> out staging [P, 2*OUT_F] for row-blocks 2q, 2q+1

            # Factor loads on the scalar HWDGE ring, x stream on the sync
            # ring -- the rings share the 16 SDMA engines at packet
            # granularity, so the small factor loads overlap the x stream.
            nc.scalar.dma_start(out=vt[:], in_=vt_d[:])
            nc.scalar.dma_start(out=u2[:], in_=u2_d[:])
            for h in range(NH):
                for G in range(NG):
                    xt[h, G] = xt_p.tile(
                        [P, KCG * RH], BF16, tag="xt", name=f"xt{h}{G}"
                    )
                    nc.sync.dma_start(out=xt[h, G][:], in_=x_d[h, G])

            def g1_pair(h, G, m, start, stop):
                # pair (kc_a, kc_b) = (16G + 2m, 16G + 2m + 1): two
                # concurrent col-tiled matmuls into the two PSUM halves.
                kc_a = KCG * G + 2 * m
                for half, kc in ((0, kc_a), (1, kc_a + 1)):
                    j = kc - KCG * G
                    nc.tensor.matmul(
                        ph[h][half * RANK : (half + 1) * RANK, :],
                        vt[:, kc * RANK : (kc + 1) * RANK],
                        xt[h, G][:, j * RH : (j + 1) * RH],
                        start=start,
                        stop=stop,
                        tile_position=(0, half * RANK),
                        skip_group_check=True,
                    )

            def g1_pass(h):
                for G in range(NG):
                    for m in range(KCG // 2):
                        g1_pair(h, G, m, start=(G == 0 and m == 0),
                                stop=(G == NG - 1 and m == KCG // 2 - 1))

            def g2_rb(h, rb):
                q, b = divmod(rb, 2)
                if b == 0:
                    ob[h, q] = obuf_p.tile(
                        [P, 2 * OUT_F], BF16, tag="ob", name=f"ob{h}{q}"
                    )
                for w in range(N_PO):
                    po = po_p.tile([P, PO_W], F32, tag="po")
                    for s in range(2):
                        nb = w * 2 + s
                        nc.tensor.matmul(
                            po[:, s * NB : (s + 1) * NB],
                            ht[h][:, rb * P : (rb + 1) * P],
                            u2[:, nb * NB : (nb + 1) * NB],
                            start=True,
                            stop=True,
                        )
                    dst = ob[h, q][:, b * OUT_F + w * PO_W : b * OUT_F + (w + 1) * PO_W]
                    if w % 2 == 0:
                        nc.vector.tensor_copy(out=dst, in_=po[:])
                    else:
                        nc.scalar.copy(out=dst, in_=po[:])
                if b == 1:
                    nc.scalar.dma_start(out=o_d[h, q], in_=ob[h, q][:])

            # ---- pass 0 GEMM1 ----
            ph[0] = ph_p.tile([P, RH], F32, tag="ph", name="ph0")
            g1_pass(0)
            ht[0] = ht_p.tile([P, RH], BF16, tag="ht", name="ht0")
            nc.vector.tensor_copy(out=ht[0][:], in_=ph[0][:])

            # ---- pass-0 GEMM2 (covers the pass-1 x DMA window) ----
            for rb in range(N_RB):
                g2_rb(0, rb)

            # ---- pass 1 GEMM1 ----
            ph[1] = ph_p.tile([P, RH], F32, tag="ph", name="ph1")
            g1_pass(1)
            ht[1] = ht_p.tile([P, RH], BF16, tag="ht", name="ht1")
            nc.vector.tensor_copy(out=ht[1][:], in_=ph[1][:])

            # ---- pass 1 GEMM2 ----
            for rb in range(N_RB):
                g2_rb(1, rb)

    return nc


_NC_CACHE = None


def _get_nc():
    global _NC_CACHE
    if _NC_CACHE is None:
        _NC_CACHE = build_bass()
        _NC_CACHE.finalize()
    return _NC_CACHE


def _pack_inputs(inputs):
    x = np.ascontiguousarray(np.asarray(inputs["x"], dtype=np.float32))
    u = np.asarray(inputs["U"], dtype=np.float32)
    v = np.asarray(inputs["V"], dtype=np.float32)

    xb = x.reshape(ROWS, IN_F).astype(BF)
    # XT[c, h, G, p, j, r] = x[c*1024 + h*512 + r, (G*16 + j)*128 + p]
    xt_host = np.ascontiguousarray(
        xb.view(np.uint16)
        .reshape(N_CORES, NH, RH, NG, KCG, P)
        .transpose(0, 1, 3, 5, 4, 2)
    ).view(BF)

    vt_host = np.ascontiguousarray(
        v.reshape(RANK, N_KC, P).transpose(2, 1, 0).reshape(P, N_KC * RANK)
    ).astype(BF)
    ut = np.ascontiguousarray(u.T).astype(BF)       # [64, 4096]
    u2_host = np.ascontiguousarray(np.concatenate([ut, ut], axis=0))  # [128, 4096]
    return xt_host, vt_host, u2_host


def run(inputs, trace=False):
    """Returns (full_output, exec_time_ns or None)."""
    xt_host, vt_host, u2_host = _pack_inputs(inputs)

    nc = _get_nc()
    core_ids = list(range(N_CORES))
    in_maps = [
        {
            "XT": xt_host[c].reshape(NH, NG, P, KCG * RH),
            "VT": vt_host,
            "U2": u2_host,
        }
        for c in core_ids
    ]
    res = run_bass_kernel_spmd(nc, in_maps, core_ids, trace=trace)
    # out[h, q, p, b, o] -> row h*512 + (2q+b)*128 + p
    out = np.concatenate(
        [
            np.asarray(r["out"]).transpose(0, 1, 3, 2, 4).reshape(ROWS_PC, OUT_F)
            for r in res.results
        ],
        axis=0,
    )
    return (
        out.astype(np.float32).reshape(BATCH, SEQ, OUT_F),
        res.exec_time_ns,
    )


def kernel(**inputs):
    return run(inputs)[0]


# revision 4
# speedup vs baseline: 1.1608x; 1.1608x over previous
"""Low-rank linear: out = x @ (U @ V)^T = (x @ V^T) @ U^T on 8 TRN2 cores.

Shapes (hardcoded per problem spec):
  x [4, 2048, 4096] f32 -> flat [8192, 4096], row-sharded 1024 rows/core
  U [4096, 64] f32 (replicated), V [64, 4096] f32 (replicated)
  out [4, 2048, 4096] f32

DMA-bound design: bf16 I/O (half the HBM bytes of f32) and no on-device
transposes -- the host packs x^T into the exact SBUF layout GEMM1 needs:
  XT[h, p, j, r] = x_core[h*256 + r, j*128 + p]   (bf16)
Per core ~16.3 MB of DMA (~42 us at the measured ~390 GB/s two-ring
rate) vs ~20 us of PE work and ~23 us of PSUM-evacuation copy work
(split DVE/ACT), so the DMA stream is the roofline.

Four 256-row passes pipeline in + compute + out so the copy work never
bunches into a tail. PE structure:
  GEMM1: col-tiled pairs -- two concurrent matmuls (tile_position (0,0)
    and (0,64)) accumulate partial sums hA = sum over even k-chunks into
    PSUM partitions 0..63 and hB = odd k-chunks into 64..127.
  GEMM2: contracts K=128 over the stacked [hA; hB] against [U^T; U^T]
    (U^T duplicated on both partition halves), so the hA+hB reduction
    happens inside the matmul -- full 128-row array utilization.
Out-DMAs + U^T ride the scalar HWDGE ring; V^T and the x stream ride
the sync ring, so the rings' transfers overlap.
"""

import sys

for p in ("/opt/trn_rl_repo",):
    if p not in sys.path:
        sys.path.insert(0, p)

import numpy as np
import ml_dtypes

import concourse.bass as bass
import concourse.bacc as bacc_mod
import concourse.mybir as mybir
import concourse.tile as tile
from concourse.bass_utils import run_bass_kernel_spmd

N_CORES = 8
BATCH, SEQ, IN_F = 4, 2048, 4096
ROWS = BATCH * SEQ           # 8192
ROWS_PC = ROWS // N_CORES    # 1024 rows per core
RANK = 64
OUT_F = 4096

P = 128                      # partition dim / k-chunk
N_KC = IN_F // P             # 32 k-chunks
NH = 4                       # row passes per core (one 2MB in-DMA each)
RH = ROWS_PC // NH           # 256 rows per pass
N_RB = RH // P               # 2 row-blocks of 128 per pass
NB = 512                     # out-feature block (one PSUM bank of fp32)
PO_W = 2 * NB                # po psum tile spans 2 banks -> 1 copy per 1024
N_PO = OUT_F // PO_W         # 4 po tiles per row-block

F32 = mybir.dt.float32
BF16 = mybir.dt.bfloat16
BF = ml_dtypes.bfloat16


def build_bass():
    nc = bacc_mod.Bacc("TRN2")
    x_d = nc.declare_dram_parameter("XT", [NH, P, N_KC * RH], BF16, isOutput=False)
    vt_d = nc.declare_dram_parameter("VT", [P, N_KC * RANK], BF16, isOutput=False)
    u2_d = nc.declare_dram_parameter("U2", [P, OUT_F], BF16, isOutput=False)
    # out[pp, p, t, o] -> row pp*512 + t*128 + p   (pass-pair pp, 2MB DMA each)
    o_d = nc.declare_dram_parameter("out", [NH // 2, P, 4, OUT_F], BF16, isOutput=True)

    with tile.TileContext(nc) as tc:
        with (
            tc.tile_pool(name="const", bufs=1) as const,
            tc.tile_pool(name="xt", bufs=3) as xt_p,
            tc.tile_pool(name="ht", bufs=2) as ht_p,
            tc.tile_pool(name="obuf", bufs=2) as obuf_p,
            tc.tile_pool(name="ph", bufs=2, space="PSUM") as ph_p,
            tc.tile_pool(name="po", bufs=3, space="PSUM") as po_p,
        ):
            vt = const.tile([P, N_KC * RANK], BF16, tag="vt")
            u2 = const.tile([P, OUT_F], BF16, tag="u2")

            xt = {}   # h -> input tile [P, N_KC*RH]
            ph = {}   # h -> GEMM1 psum [P, RH]: rows 0..63 hA, 64..127 hB
            ht = {}   # h -> [hA; hB] in SBUF bf16 [P, RH]
            ob = {}   # pp -> out staging [P, 4*OUT_F] for row-blocks 4pp..4pp+3

            # V^T first on the sync ring (needed by the first matmul),
            # then the x stream; U^T + out-DMAs ride the scalar ring.
            nc.sync.dma_start(out=vt[:], in_=vt_d[:])
            nc.scalar.dma_start(out=u2[:], in_=u2_d[:])
            for h in range(NH):
                xt[h] = xt_p.tile([P, N_KC * RH], BF16, tag="xt", name=f"xt{h}")
                nc.sync.dma_start(out=xt[h][:], in_=x_d[h])

            def g1_pass(h):
                # pair m = (kc 2m, kc 2m+1): two concurrent col-tiled
                # matmuls into the two PSUM partition halves.
                for m in range(N_KC // 2):
                    for half in range(2):
                        kc = 2 * m + half
                        nc.tensor.matmul(
                            ph[h][half * RANK : (half + 1) * RANK, :],
                            vt[:, kc * RANK : (kc + 1) * RANK],
                            xt[h][:, kc * RH : (kc + 1) * RH],
                            start=(m == 0),
                            stop=(m == N_KC // 2 - 1),
                            tile_position=(0, half * RANK),
                            skip_group_check=True,
                        )

            def g2_rb(h, rb):
                pp = h // 2
                t = 2 * (h % 2) + rb
                for w in range(N_PO):
                    po = po_p.tile([P, PO_W], F32, tag="po")
                    for s in range(2):
                        nb = w * 2 + s
                        nc.tensor.matmul(
                            po[:, s * NB : (s + 1) * NB],
                            ht[h][:, rb * P : (rb + 1) * P],
                            u2[:, nb * NB : (nb + 1) * NB],
                            start=True,
                            stop=True,
                        )
                    dst = ob[pp][:, t * OUT_F + w * PO_W : t * OUT_F + (w + 1) * PO_W]
                    if w % 2 == 0:
                        nc.vector.tensor_copy(out=dst, in_=po[:])
                    else:
                        nc.scalar.copy(out=dst, in_=po[:])

            for h in range(NH):
                ph[h] = ph_p.tile([P, RH], F32, tag="ph", name=f"ph{h}")
                g1_pass(h)
                ht[h] = ht_p.tile([P, RH], BF16, tag="ht", name=f"ht{h}")
                nc.vector.tensor_copy(out=ht[h][:], in_=ph[h][:])
                if h % 2 == 0:
                    ob[h // 2] = obuf_p.tile(
                        [P, 4 * OUT_F], BF16, tag="ob", name=f"ob{h // 2}"
                    )
                for rb in range(N_RB):
                    g2_rb(h, rb)
                if h % 2 == 1:
                    nc.scalar.dma_start(out=o_d[h // 2], in_=ob[h // 2][:])

    return nc


_NC_CACHE = None


def _get_nc():
    global _NC_CACHE
    if _NC_CACHE is None:
        _NC_CACHE = build_bass()
        _NC_CACHE.finalize()
    return _NC_CACHE


def _pack_inputs(inputs):
    x = np.ascontiguousarray(np.asarray(inputs["x"], dtype=np.float32))
    u = np.asarray(inputs["U"], dtype=np.float32)
    v = np.asarray(inputs["V"], dtype=np.float32)

    xb = x.reshape(ROWS, IN_F).astype(BF)
    # XT[c, h, p, j, r] = x[c*1024 + h*256 + r, j*128 + p]
    xt_host = np.ascontiguousarray(
        xb.view(np.uint16)
        .reshape(N_CORES, NH, RH, N_KC, P)
        .transpose(0, 1, 4, 3, 2)
    ).view(BF)

    vt_host = np.ascontiguousarray(
        v.reshape(RANK, N_KC, P).transpose(2, 1, 0).reshape(P, N_KC * RANK)
    ).astype(BF)
    ut = np.ascontiguousarray(u.T).astype(BF)       # [64, 4096]
    u2_host = np.ascontiguousarray(np.concatenate([ut, ut], axis=0))  # [128, 4096]
    return xt_host, vt_host, u2_host


def run(inputs, trace=False):
    """Returns (full_output, exec_time_ns or None)."""
    xt_host, vt_host, u2_host = _pack_inputs(inputs)

    nc = _get_nc()
    core_ids = list(range(N_CORES))
    in_maps = [
        {
            "XT": xt_host[c].reshape(NH, P, N_KC * RH),
            "VT": vt_host,
            "U2": u2_host,
        }
        for c in core_ids
    ]
    res = run_bass_kernel_spmd(nc, in_maps, core_ids, trace=trace)
    # out[pp, p, t, o] -> row pp*512 + t*128 + p
    out = np.concatenate(
        [
            np.asarray(r["out"]).transpose(0, 2, 1, 3).reshape(ROWS_PC, OUT_F)
            for r in res.results
        ],
        axis=0,
    )
    return (
        out.astype(np.float32).reshape(BATCH, SEQ, OUT_F),
        res.exec_time_ns,
    )


def kernel(**inputs):
    return run(inputs)[0]
